# revision 4
# baseline (speedup 1.0000x reference)
"""Trainium2 Bass kernel for nn_CrossAttention (3x3 scale-grid cross attention).

Reference computation (per batch b):
    WV_i = V_i @ W.T + b                    (video projection, i in 0..2)
    S_ij = (WV_i @ A_j.T) / sqrt(C)         [T, S] scores
    P_ij = softmax(S_ij, axis=-1)
    fv[i,j] = P_ij @ A_j        -> out[0, i, j, b]
    fa[j,i] = P_ij.T @ V_i      -> out[1, j, i, b]

Sharding: data-parallel over batch B=8 across the 8 NeuronCores (one batch
element per core). W/b replicated. Each core runs all 9 (i,j) attention pairs
for its batch element.

On-chip plan (per core): fp8e4m3 DoubleRow matmuls (contract 256 per pass)
for the three big GEMM families (scores, fv, fa); fp32 PSUM accumulation.
The WV projection stays bf16 (its stationary W is precision-critical).
Softmax: P is stored as exp(S/sqrt(C) - ln8) in fp8 (bias keeps the max
under fp8e4m3's 240 limit; the 1/8 cancels in both outputs). Row sums are
accumulated in f32 by the ACT Exp pass; fv is normalized at the PSUM->SBUF
copy, fa via a row-scaled copy of V (scaled x128 before fp8 quantization to
dodge subnormal underflow, divided back at the PSUM drain). P^T for the fv
stationary is produced by PE transposes pipelined into the scores phase.
DoubleRow chunk pairs live in the same partitions at different free offsets:
A/AT/WV^T/P/P^T/Vr are each one wide SBUF tile [128, nchunks*stride].
"""

import numpy as np
from contextlib import ExitStack

import ml_dtypes

import concourse.bacc as bacc
import concourse.bass as bass
import concourse.mybir as mybir
import concourse.tile as tile
from concourse.bass_utils import run_bass_kernel_spmd
from concourse.masks import make_identity

BF16 = mybir.dt.bfloat16
F8 = mybir.dt.float8e4
F32 = mybir.dt.float32
AF = mybir.ActivationFunctionType
DR = mybir.MatmulPerfMode.DoubleRow

B, T, C = 8, 1024, 512
P = 128
NT = T // P   # 8 row blocks
ND = C // P   # 4 feature chunks
SCALE = 1.0 / float(np.sqrt(C))
EXP_BIAS = -float(np.log(8.0))   # keep exp() under fp8e4m3 max (240)
VR_SCALE = 128.0                 # lift V*recip out of fp8 subnormal range

_CACHE = {}


def _build(repeat=1):
    key = ("nc", repeat)
    if key in _CACHE:
        return _CACHE[key]

    nc = bacc.Bacc("TRN2", target_bir_lowering=False, debug=False, num_devices=8)

    a_dram = [nc.dram_tensor(f"a{j}", [T, C], F8, kind="ExternalInput").ap()
              for j in range(3)]
    v_dram = [nc.dram_tensor(f"v{i}", [T, C], BF16, kind="ExternalInput").ap()
              for i in range(3)]
    at_dram = [nc.dram_tensor(f"at{j}", [C, T], F8, kind="ExternalInput").ap()
               for j in range(3)]
    vt_dram = [nc.dram_tensor(f"vt{i}", [C, T], BF16, kind="ExternalInput").ap()
               for i in range(3)]
    wt_dram = nc.dram_tensor("WT", [C, C], BF16, kind="ExternalInput").ap()
    b_dram = nc.dram_tensor("bvec", [ND, P, 1], F32, kind="ExternalInput").ap()
    c_dram = nc.dram_tensor("cvec", [2, P, 1], F32, kind="ExternalInput").ap()
    out_dram = nc.dram_tensor("out", [2, 3, 3, T, C], F32, kind="ExternalOutput").ap()

    with ExitStack() as ctx:
        tc = ctx.enter_context(tile.TileContext(nc))

        const = ctx.enter_context(tc.tile_pool(name="const", bufs=1))
        big = ctx.enter_context(tc.tile_pool(name="big", bufs=1))
        work = ctx.enter_context(tc.tile_pool(name="work", bufs=1))
        small = ctx.enter_context(tc.tile_pool(name="small", bufs=1))
        stage = ctx.enter_context(tc.tile_pool(name="stage", bufs=8))

        ps_s = ctx.enter_context(tc.tile_pool(name="ps_s", bufs=2, space="PSUM"))
        ps_t = ctx.enter_context(tc.tile_pool(name="ps_t", bufs=2, space="PSUM"))
        ps_o = ctx.enter_context(tc.tile_pool(name="ps_o", bufs=2, space="PSUM"))

        ident = const.tile([P, P], F8, tag="ident", name="ident")
        make_identity(nc, ident[:])

        for _rep in range(repeat):
            _kernel_body(nc, tc, const, big, work, small, stage,
                         ps_s, ps_t, ps_o, ident,
                         a_dram, v_dram, at_dram, vt_dram, wt_dram, b_dram,
                         c_dram, out_dram)

    nc.compile()
    _CACHE[key] = nc
    return nc


def _kernel_body(nc, tc, const, big, work, small, stage,
                 ps_s, ps_t, ps_o, ident,
                 a_dram, v_dram, at_dram, vt_dram, wt_dram, b_dram, c_dram,
                 out_dram):
    # ---- load operands (transposed copies prepared on host) ----
    # Issue order = startup critical path: the first WV matmul needs all of
    # WT plus VT[0]; scores pair (0,0) then needs ATa[0]; fa needs Vbf[0].
    WT = [big.tile([P, C], BF16, tag=f"WT{cc}", name=f"WT{cc}")
          for cc in range(ND)]
    for cc in range(ND):
        nc.sync.dma_start(WT[cc][:], wt_dram[cc * P:(cc + 1) * P, :])

    VT = [[big.tile([P, T], BF16, tag=f"VT{i}_{cc}", name=f"VT{i}_{cc}")
           for cc in range(ND)] for i in range(3)]
    for cc in range(ND):
        nc.sync.dma_start(VT[0][cc][:], vt_dram[0][cc * P:(cc + 1) * P, :])

    b_sb = []
    for dc in range(ND):
        t_ = const.tile([P, 1], F32, tag=f"b{dc}", name=f"b{dc}")
        nc.sync.dma_start(t_[:], b_dram[dc])
        b_sb.append(t_)
    c128 = const.tile([P, 1], F32, tag="c128", name="c128")
    nc.sync.dma_start(c128[:], c_dram[0])
    c1_128 = const.tile([P, 1], F32, tag="c1_128", name="c1_128")
    nc.sync.dma_start(c1_128[:], c_dram[1])

    # fp8 A in both layouts; chunk index lives in the free dim so DoubleRow
    # pairs (2g, 2g+1) share partitions.
    ATa = [big.tile([P, ND * T], F8, tag=f"ATa{j}", name=f"ATa{j}")
           for j in range(3)]
    for dc in range(ND):
        nc.sync.dma_start(ATa[0][:, dc * T:(dc + 1) * T],
                          at_dram[0][dc * P:(dc + 1) * P, :])

    Vbf = [[big.tile([P, C], BF16, tag=f"V{i}_{tb}", name=f"V{i}_{tb}")
            for tb in range(NT)] for i in range(3)]
    for tb in range(NT):
        nc.sync.dma_start(Vbf[0][tb][:], v_dram[0][tb * P:(tb + 1) * P, :])

    for i in range(1, 3):
        for cc in range(ND):
            nc.sync.dma_start(VT[i][cc][:], vt_dram[i][cc * P:(cc + 1) * P, :])
    for j in range(1, 3):
        for dc in range(ND):
            nc.sync.dma_start(ATa[j][:, dc * T:(dc + 1) * T],
                              at_dram[j][dc * P:(dc + 1) * P, :])

    Aa = [big.tile([P, NT * C], F8, tag=f"Aa{j}", name=f"Aa{j}")
          for j in range(3)]
    for sc in range(NT):
        nc.sync.dma_start(Aa[0][:, sc * C:(sc + 1) * C],
                          a_dram[0][sc * P:(sc + 1) * P, :])
    for i in range(1, 3):
        for tb in range(NT):
            nc.sync.dma_start(Vbf[i][tb][:], v_dram[i][tb * P:(tb + 1) * P, :])
    for j in range(1, 3):
        for sc in range(NT):
            nc.sync.dma_start(Aa[j][:, sc * C:(sc + 1) * C],
                              a_dram[j][sc * P:(sc + 1) * P, :])

    # ---- WV^T_i[d, t] = W^T @ V^T_i + b (bf16 matmul, fp8 out) ----
    # cc outer / th inner: consecutive matmuls share the stationary operand.
    Wv8 = [big.tile([P, ND * T], F8, tag=f"Wv8{i}", name=f"Wv8{i}")
           for i in range(3)]
    for i in range(3):
        for dc in range(ND):
            po2 = [ps_o.tile([P, C], F32, tag="o", name="o") for _ in range(2)]
            for cc in range(ND):
                for th in range(2):
                    nc.tensor.matmul(po2[th][:], WT[cc][:, dc * P:(dc + 1) * P],
                                     VT[i][cc][:, th * C:(th + 1) * C],
                                     start=(cc == 0), stop=(cc == ND - 1))
            for th in range(2):
                nc.scalar.activation(
                    Wv8[i][:, dc * T + th * C: dc * T + (th + 1) * C],
                    po2[th][:], AF.Identity, bias=b_sb[dc][:], scale=1.0)

    # ---- main loop over the 9 attention pairs ----
    for i in range(3):
        for j in range(3):
            Pa = work.tile([P, NT * T], F8, tag="Pa", name="Pa")
            Par = Pa.rearrange("p (tb s) -> p tb s", tb=NT)
            PTa = work.tile([P, NT * T], F8, tag="PTa", name="PTa")
            PTr = PTa.rearrange("p (sc t) -> p sc t", sc=NT)
            recip = [small.tile([P, 1], F32, tag=f"rc{tb}", name=f"rc{tb}")
                     for tb in range(NT)]
            rc128 = [small.tile([P, 1], F32, tag=f"rk{tb}", name=f"rk{tb}")
                     for tb in range(NT)]
            Vra = work.tile([P, NT * C], F8, tag="Vra", name="Vra")
            Vrr = Vra.rearrange("p (tb c) -> p tb c", tb=NT)

            Wv8r = Wv8[i].rearrange("p (dc t) -> p dc t", dc=ND)
            ATr = ATa[j].rearrange("p (dc s) -> p dc s", dc=ND)
            Ar = Aa[j].rearrange("p (sc c) -> p sc c", sc=NT)

            def transpose_block(tb):
                # 8 transposed blocks of P[tb] -> one PSUM bank -> strided copy
                pt = ps_t.tile([P, T], F8, tag="t", name="t")
                for sc in range(NT):
                    nc.tensor.transpose(
                        pt[:, sc * P:(sc + 1) * P],
                        Pa[:, tb * T + sc * P: tb * T + (sc + 1) * P], ident[:])
                nc.vector.tensor_copy(
                    PTr[:, :, tb * P:(tb + 1) * P],
                    pt[:].rearrange("p (sc t) -> p sc t", sc=NT))

            for tb in range(NT):
                # one [128, 1024] score block = 2 PSUM banks; each DoubleRow
                # matmul contracts 256 and stays within one bank. g outer /
                # h inner shares the stationary operand between the halves.
                ps = ps_s.tile([P, T], F32, tag="s", name="s")
                for g in range(2):
                    for h in range(2):
                        nc.tensor.matmul(
                            ps[:, h * C:(h + 1) * C],
                            Wv8r[:, 2 * g:2 * g + 2, tb * P:(tb + 1) * P],
                            ATr[:, 2 * g:2 * g + 2, h * C:(h + 1) * C],
                            start=(g == 0), stop=(g == 1), perf_mode=DR)
                rsum = small.tile([P, 1], F32, tag=f"rsum{tb}", name=f"rsum{tb}")
                nc.scalar.activation(Pa[:, tb * T:(tb + 1) * T], ps[:], AF.Exp,
                                     scale=SCALE, bias=EXP_BIAS,
                                     accum_out=rsum[:])
                nc.vector.reciprocal(recip[tb][:], rsum[:])
                nc.vector.tensor_scalar_mul(rc128[tb][:], recip[tb][:], c128[:])
                nc.vector.tensor_scalar_mul(Vrr[:, tb, :], Vbf[i][tb][:],
                                            rc128[tb][:])
                if tb >= 1:
                    transpose_block(tb - 1)
            transpose_block(NT - 1)

            # fa[j,i] = P_raw^T @ (diag(128*recip) @ V_i) / 128
            # fv[i,j] = diag(recip) @ (P_raw @ A_j)
            # interleaved so the kernel tail drains two engines in parallel
            for k in range(NT):
                po = ps_o.tile([P, C], F32, tag="o", name="o")
                for g in range(NT // 2):
                    nc.tensor.matmul(
                        po[:],
                        Par[:, 2 * g:2 * g + 2, k * P:(k + 1) * P],
                        Vrr[:, 2 * g:2 * g + 2, :],
                        start=(g == 0), stop=(g == NT // 2 - 1), perf_mode=DR)
                st = stage.tile([P, C], F32, tag="st", name="st")
                nc.vector.tensor_scalar_mul(st[:], po[:], c1_128[:])
                nc.sync.dma_start(out_dram[1, j, i, k * P:(k + 1) * P, :], st[:])

                po = ps_o.tile([P, C], F32, tag="o", name="o")
                for g in range(NT // 2):
                    nc.tensor.matmul(
                        po[:],
                        PTr[:, 2 * g:2 * g + 2, k * P:(k + 1) * P],
                        Ar[:, 2 * g:2 * g + 2, :],
                        start=(g == 0), stop=(g == NT // 2 - 1), perf_mode=DR)
                st = stage.tile([P, C], F32, tag="st", name="st")
                nc.scalar.activation(st[:], po[:], AF.Copy, bias=0.0,
                                     scale=recip[k][:])
                nc.sync.dma_start(out_dram[0, i, j, k * P:(k + 1) * P, :], st[:])


def _prep_in_maps(a0, a1, a2, v0, v1, v2, W, b):
    bf = ml_dtypes.bfloat16
    f8 = ml_dtypes.float8_e4m3
    a_f32 = [np.asarray(x, dtype=np.float32) for x in (a0, a1, a2)]
    v_bf = [np.asarray(x, dtype=np.float32).astype(bf) for x in (v0, v1, v2)]
    a_f8 = [x.astype(f8) for x in a_f32]
    wt_bf = np.ascontiguousarray(np.asarray(W, dtype=np.float32).astype(bf).T)
    b_r = np.ascontiguousarray(
        np.asarray(b, dtype=np.float32).reshape(ND, P, 1))
    c_r = np.empty((2, P, 1), dtype=np.float32)
    c_r[0] = VR_SCALE
    c_r[1] = 1.0 / VR_SCALE
    in_maps = []
    for bi in range(B):
        m = {f"a{j}": np.ascontiguousarray(a_f8[j][bi]) for j in range(3)}
        m.update({f"v{i}": np.ascontiguousarray(v_bf[i][bi]) for i in range(3)})
        m.update({f"at{j}": np.ascontiguousarray(a_f32[j][bi].T).astype(f8)
                  for j in range(3)})
        m.update({f"vt{i}": np.ascontiguousarray(v_bf[i][bi].T)
                  for i in range(3)})
        m["WT"] = wt_bf
        m["bvec"] = b_r
        m["cvec"] = c_r
        in_maps.append(m)
    return in_maps


def run(inputs, trace=False, tmpdir=None):
    """Build+run on 8 cores; returns (full_output, BassKernelResults)."""
    nc = _build()
    in_maps = _prep_in_maps(**inputs)
    res = run_bass_kernel_spmd(nc, in_maps, list(range(B)), trace=trace,
                               tmpdir=tmpdir)
    out = np.empty((2, 3, 3, B, T, C), dtype=np.float32)
    for bi in range(B):
        out[:, :, :, bi] = res.results[bi]["out"]
    return out, res


def kernel(a0, a1, a2, v0, v1, v2, W, b):
    out, _ = run(dict(a0=a0, a1=a1, a2=a2, v0=v0, v1=v1, v2=v2, W=W, b=b))
    return out


# revision 8
# speedup vs baseline: 70.7893x; 70.7893x over previous
"""Trainium2 Bass kernel for nn_CrossAttention (3x3 scale-grid cross attention).

Reference computation (per batch b):
    WV_i = V_i @ W.T + b                    (video projection, i in 0..2)
    S_ij = (WV_i @ A_j.T) / sqrt(C)         [T, S] scores
    P_ij = softmax(S_ij, axis=-1)
    fv[i,j] = P_ij @ A_j        -> out[0, i, j, b]
    fa[j,i] = P_ij.T @ V_i      -> out[1, j, i, b]

Sharding: data-parallel over batch B=8 across the 8 NeuronCores (one batch
element per core). W/b replicated. Each core runs all 9 (i,j) attention pairs
for its batch element.

On-chip plan (per core): bf16 matmul paths with fp32 PSUM accumulation.
Softmax normalization is folded into the outputs (fv scaled at the PSUM->SBUF
copy, fa via a row-scaled copy of V). The host pre-transposes A/V/W (layout
prep only) so the device only transposes P. P^T transposes are software-
pipelined into the scores phase: the 8 transposed blocks of each P row-block
go to one PSUM bank and leave via one strided DVE copy.
"""

import numpy as np
from contextlib import ExitStack

import ml_dtypes

import concourse.bacc as bacc
import concourse.bass as bass
import concourse.mybir as mybir
import concourse.tile as tile
from concourse.bass_utils import run_bass_kernel_spmd
from concourse.masks import make_identity

BF16 = mybir.dt.bfloat16
F32 = mybir.dt.float32
AF = mybir.ActivationFunctionType

B, T, C = 8, 1024, 512
P = 128
NT = T // P   # 8 row blocks
ND = C // P   # 4 feature chunks
SCALE = 1.0 / float(np.sqrt(C))

_CACHE = {}

# Consecutive score matmuls share the stationary operand (fewer LDWEIGHTS)
# at the cost of alternating PSUM banks between back-to-back matmuls.
# Measured on HW: bank alternation costs ~2% — keep sequential.
_SCORES_BANK_INTERLEAVE = False


def _build(repeat=1):
    key = ("nc", repeat, _SCORES_BANK_INTERLEAVE)
    if key in _CACHE:
        return _CACHE[key]

    nc = bacc.Bacc("TRN2", target_bir_lowering=False, debug=False, num_devices=8)

    a_dram = [nc.dram_tensor(f"a{j}", [T, C], BF16, kind="ExternalInput").ap()
              for j in range(3)]
    v_dram = [nc.dram_tensor(f"v{i}", [T, C], BF16, kind="ExternalInput").ap()
              for i in range(3)]
    at_dram = [nc.dram_tensor(f"at{j}", [C, T], BF16, kind="ExternalInput").ap()
               for j in range(3)]
    vt_dram = [nc.dram_tensor(f"vt{i}", [C, T], BF16, kind="ExternalInput").ap()
               for i in range(3)]
    wt_dram = nc.dram_tensor("WT", [C, C], BF16, kind="ExternalInput").ap()
    b_dram = nc.dram_tensor("bvec", [ND, P, 1], F32, kind="ExternalInput").ap()
    out_dram = nc.dram_tensor("out", [2, 3, 3, T, C], F32, kind="ExternalOutput").ap()

    with ExitStack() as ctx:
        tc = ctx.enter_context(tile.TileContext(nc))

        const = ctx.enter_context(tc.tile_pool(name="const", bufs=1))
        big = ctx.enter_context(tc.tile_pool(name="big", bufs=1))
        work = ctx.enter_context(tc.tile_pool(name="work", bufs=1))
        small = ctx.enter_context(tc.tile_pool(name="small", bufs=1))
        stage = ctx.enter_context(tc.tile_pool(name="stage", bufs=4))

        ps_s = ctx.enter_context(tc.tile_pool(name="ps_s", bufs=2, space="PSUM"))
        ps_t = ctx.enter_context(tc.tile_pool(name="ps_t", bufs=2, space="PSUM"))
        ps_o = ctx.enter_context(tc.tile_pool(name="ps_o", bufs=2, space="PSUM"))

        ident = const.tile([P, P], BF16, tag="ident", name="ident")
        make_identity(nc, ident[:])

        for _rep in range(repeat):
            _kernel_body(nc, tc, const, big, work, small, stage,
                         ps_s, ps_t, ps_o, ident,
                         a_dram, v_dram, at_dram, vt_dram, wt_dram, b_dram,
                         out_dram)

    nc.compile()
    _CACHE[("nc", repeat, _SCORES_BANK_INTERLEAVE)] = nc
    return nc


def _kernel_body(nc, tc, const, big, work, small, stage,
                 ps_s, ps_t, ps_o, ident,
                 a_dram, v_dram, at_dram, vt_dram, wt_dram, b_dram, out_dram):
    # ---- load operands (bf16; transposed copies prepared on host) ----
    # Issue order = startup critical path: the first WV matmul needs all of
    # WT plus VT[0]; put those 8 tiles at the head of the DMA queues.
    WT = [big.tile([P, C], BF16, tag=f"WT{cc}", name=f"WT{cc}")
          for cc in range(ND)]
    for cc in range(ND):
        nc.sync.dma_start(WT[cc][:], wt_dram[cc * P:(cc + 1) * P, :])

    VT = [[big.tile([P, T], BF16, tag=f"VT{i}_{cc}", name=f"VT{i}_{cc}")
           for cc in range(ND)] for i in range(3)]
    AT = [[big.tile([P, T], BF16, tag=f"AT{j}_{cc}", name=f"AT{j}_{cc}")
           for cc in range(ND)] for j in range(3)]
    for cc in range(ND):
        nc.sync.dma_start(VT[0][cc][:], vt_dram[0][cc * P:(cc + 1) * P, :])

    b_sb = []
    for dc in range(ND):
        t_ = const.tile([P, 1], F32, tag=f"b{dc}", name=f"b{dc}")
        nc.sync.dma_start(t_[:], b_dram[dc])
        b_sb.append(t_)

    for i in range(1, 3):
        for cc in range(ND):
            nc.sync.dma_start(VT[i][cc][:], vt_dram[i][cc * P:(cc + 1) * P, :])
    for j in range(3):
        for cc in range(ND):
            nc.sync.dma_start(AT[j][cc][:], at_dram[j][cc * P:(cc + 1) * P, :])

    Abf = [[big.tile([P, C], BF16, tag=f"A{j}_{tb}", name=f"A{j}_{tb}")
            for tb in range(NT)] for j in range(3)]
    Vbf = [[big.tile([P, C], BF16, tag=f"V{i}_{tb}", name=f"V{i}_{tb}")
            for tb in range(NT)] for i in range(3)]
    for j in range(3):
        for tb in range(NT):
            nc.sync.dma_start(Abf[j][tb][:], a_dram[j][tb * P:(tb + 1) * P, :])
    for i in range(3):
        for tb in range(NT):
            nc.sync.dma_start(Vbf[i][tb][:], v_dram[i][tb * P:(tb + 1) * P, :])

    # ---- WV^T_i[d, t] = W^T @ V^T_i + b (bf16 out, bias folded in) ----
    # cc outer / th inner: consecutive matmuls share the stationary operand,
    # halving LDWEIGHTS traffic (the two halves accumulate in two banks).
    WVT = [[big.tile([P, T], BF16, tag=f"WVT{i}_{dc}", name=f"WVT{i}_{dc}")
            for dc in range(ND)] for i in range(3)]
    for i in range(3):
        for dc in range(ND):
            po2 = [ps_o.tile([P, C], F32, tag="o", name="o") for _ in range(2)]
            for cc in range(ND):
                for th in range(2):
                    nc.tensor.matmul(po2[th][:], WT[cc][:, dc * P:(dc + 1) * P],
                                     VT[i][cc][:, th * C:(th + 1) * C],
                                     start=(cc == 0), stop=(cc == ND - 1))
            for th in range(2):
                nc.scalar.activation(WVT[i][dc][:, th * C:(th + 1) * C],
                                     po2[th][:], AF.Identity,
                                     bias=b_sb[dc][:], scale=1.0)

    # ---- main loop over the 9 attention pairs ----
    for i in range(3):
        for j in range(3):
            Pt = [work.tile([P, T], BF16, tag=f"P{tb}", name=f"P{tb}")
                  for tb in range(NT)]
            PTa = work.tile([P, NT * T], BF16, tag="PTall", name="PTall")
            PTv = PTa.rearrange("p (sc t) -> p sc t", sc=NT)
            recip = [small.tile([P, 1], F32, tag=f"rc{tb}", name=f"rc{tb}")
                     for tb in range(NT)]
            Vr = [work.tile([P, C], BF16, tag=f"Vr{tb}", name=f"Vr{tb}")
                  for tb in range(NT)]

            def transpose_block(tb):
                # 8 transposed blocks of P[tb] -> one PSUM bank -> strided copy
                pt = ps_t.tile([P, T], BF16, tag="t", name="t")
                for sc in range(NT):
                    nc.tensor.transpose(pt[:, sc * P:(sc + 1) * P],
                                        Pt[tb][:, sc * P:(sc + 1) * P], ident[:])
                nc.vector.tensor_copy(
                    PTv[:, :, tb * P:(tb + 1) * P],
                    pt[:].rearrange("p (sc t) -> p sc t", sc=NT))

            for tb in range(NT):
                # one [128, 1024] score block = 2 PSUM banks; each matmul
                # stays within one bank. dc outer / h inner shares the
                # stationary operand between consecutive matmuls.
                ps = ps_s.tile([P, T], F32, tag="s", name="s")
                if _SCORES_BANK_INTERLEAVE:
                    for dc in range(ND):
                        for h in range(2):
                            nc.tensor.matmul(ps[:, h * C:(h + 1) * C],
                                             WVT[i][dc][:, tb * P:(tb + 1) * P],
                                             AT[j][dc][:, h * C:(h + 1) * C],
                                             start=(dc == 0), stop=(dc == ND - 1))
                else:
                    for h in range(2):
                        for dc in range(ND):
                            nc.tensor.matmul(ps[:, h * C:(h + 1) * C],
                                             WVT[i][dc][:, tb * P:(tb + 1) * P],
                                             AT[j][dc][:, h * C:(h + 1) * C],
                                             start=(dc == 0), stop=(dc == ND - 1))
                rsum = small.tile([P, 1], F32, tag=f"rsum{tb}", name=f"rsum{tb}")
                nc.scalar.activation(Pt[tb][:], ps[:], AF.Exp, scale=SCALE,
                                     accum_out=rsum[:])
                nc.vector.reciprocal(recip[tb][:], rsum[:])
                nc.vector.tensor_scalar_mul(Vr[tb][:], Vbf[i][tb][:],
                                            recip[tb][:])
                if tb >= 1:
                    transpose_block(tb - 1)
            transpose_block(NT - 1)

            # fa[j,i] = P_raw^T @ (diag(recip) @ V_i)
            # fv[i,j] = diag(recip) @ (P_raw @ A_j)
            # interleaved so the kernel tail drains two engines in parallel
            for k in range(NT):
                po = ps_o.tile([P, C], F32, tag="o", name="o")
                for tb in range(NT):
                    nc.tensor.matmul(po[:], Pt[tb][:, k * P:(k + 1) * P],
                                     Vr[tb][:],
                                     start=(tb == 0), stop=(tb == NT - 1))
                st = stage.tile([P, C], F32, tag="st", name="st")
                nc.vector.tensor_copy(st[:], po[:])
                nc.sync.dma_start(out_dram[1, j, i, k * P:(k + 1) * P, :], st[:])

                po = ps_o.tile([P, C], F32, tag="o", name="o")
                for sc in range(NT):
                    nc.tensor.matmul(
                        po[:], PTa[:, sc * T + k * P: sc * T + (k + 1) * P],
                        Abf[j][sc][:],
                        start=(sc == 0), stop=(sc == NT - 1))
                st = stage.tile([P, C], F32, tag="st", name="st")
                nc.scalar.activation(st[:], po[:], AF.Copy, bias=0.0,
                                     scale=recip[k][:])
                nc.sync.dma_start(out_dram[0, i, j, k * P:(k + 1) * P, :], st[:])


def _prep_in_maps(a0, a1, a2, v0, v1, v2, W, b):
    bf = ml_dtypes.bfloat16
    a_bf = [np.asarray(x, dtype=np.float32).astype(bf) for x in (a0, a1, a2)]
    v_bf = [np.asarray(x, dtype=np.float32).astype(bf) for x in (v0, v1, v2)]
    wt_bf = np.ascontiguousarray(np.asarray(W, dtype=np.float32).astype(bf).T)
    b_r = np.ascontiguousarray(
        np.asarray(b, dtype=np.float32).reshape(ND, P, 1))
    in_maps = []
    for bi in range(B):
        m = {f"a{j}": np.ascontiguousarray(a_bf[j][bi]) for j in range(3)}
        m.update({f"v{i}": np.ascontiguousarray(v_bf[i][bi]) for i in range(3)})
        m.update({f"at{j}": np.ascontiguousarray(a_bf[j][bi].T)
                  for j in range(3)})
        m.update({f"vt{i}": np.ascontiguousarray(v_bf[i][bi].T)
                  for i in range(3)})
        m["WT"] = wt_bf
        m["bvec"] = b_r
        in_maps.append(m)
    return in_maps


def run(inputs, trace=False, tmpdir=None):
    """Build+run on 8 cores; returns (full_output, BassKernelResults)."""
    nc = _build()
    in_maps = _prep_in_maps(**inputs)
    res = run_bass_kernel_spmd(nc, in_maps, list(range(B)), trace=trace,
                               tmpdir=tmpdir)
    out = np.empty((2, 3, 3, B, T, C), dtype=np.float32)
    for bi in range(B):
        out[:, :, :, bi] = res.results[bi]["out"]
    return out, res


def kernel(a0, a1, a2, v0, v1, v2, W, b):
    out, _ = run(dict(a0=a0, a1=a1, a2=a2, v0=v0, v1=v1, v2=v2, W=W, b=b))
    return out



# revision 9
# speedup vs baseline: 422.9637x; 5.9750x over previous
"""Trainium2 Bass kernel for nn_CrossAttention (3x3 scale-grid cross attention).

Reference computation (per batch b):
    WV_i = V_i @ W.T + b                    (video projection, i in 0..2)
    S_ij = (WV_i @ A_j.T) / sqrt(C)         [T, S] scores
    P_ij = softmax(S_ij, axis=-1)
    fv[i,j] = P_ij @ A_j        -> out[0, i, j, b]
    fa[j,i] = P_ij.T @ V_i      -> out[1, j, i, b]

Sharding: data-parallel over batch B=8 across the 8 NeuronCores (one batch
element per core). W/b replicated. Each core runs all 9 (i,j) attention pairs
for its batch element.

On-chip plan (per core): bf16 matmul paths with fp32 PSUM accumulation.
Softmax normalization is folded into the outputs (fv scaled at the PSUM->SBUF
copy, fa via a row-scaled copy of V). The host pre-transposes A/V/W (layout
prep only) so the device only transposes P. P^T transposes are software-
pipelined into the scores phase: the 8 transposed blocks of each P row-block
go to one PSUM bank and leave via one strided DVE copy.
"""

import numpy as np
from contextlib import ExitStack

import ml_dtypes

import concourse.bacc as bacc
import concourse.bass as bass
import concourse.mybir as mybir
import concourse.tile as tile
from concourse.bass_utils import run_bass_kernel_spmd
from concourse.masks import make_identity

BF16 = mybir.dt.bfloat16
F32 = mybir.dt.float32
AF = mybir.ActivationFunctionType

B, T, C = 8, 1024, 512
P = 128
NT = T // P   # 8 row blocks
ND = C // P   # 4 feature chunks
SCALE = 1.0 / float(np.sqrt(C))

_CACHE = {}

# Consecutive score matmuls share the stationary operand (fewer LDWEIGHTS)
# at the cost of alternating PSUM banks between back-to-back matmuls.
# Measured on HW: bank alternation costs ~2% — keep sequential.
_SCORES_BANK_INTERLEAVE = False


def _build(repeat=1):
    key = ("nc", repeat, _SCORES_BANK_INTERLEAVE)
    if key in _CACHE:
        return _CACHE[key]

    nc = bacc.Bacc("TRN2", target_bir_lowering=False, debug=False, num_devices=8)

    a_dram = [nc.dram_tensor(f"a{j}", [T, C], BF16, kind="ExternalInput").ap()
              for j in range(3)]
    v_dram = [nc.dram_tensor(f"v{i}", [T, C], BF16, kind="ExternalInput").ap()
              for i in range(3)]
    at_dram = [nc.dram_tensor(f"at{j}", [C, T], BF16, kind="ExternalInput").ap()
               for j in range(3)]
    vt_dram = [nc.dram_tensor(f"vt{i}", [C, T], BF16, kind="ExternalInput").ap()
               for i in range(3)]
    wt_dram = nc.dram_tensor("WT", [C, C], BF16, kind="ExternalInput").ap()
    b_dram = nc.dram_tensor("bvec", [ND, P, 1], F32, kind="ExternalInput").ap()
    out_dram = nc.dram_tensor("out", [2, 3, 3, T, C], BF16, kind="ExternalOutput").ap()

    with ExitStack() as ctx:
        tc = ctx.enter_context(tile.TileContext(nc))

        const = ctx.enter_context(tc.tile_pool(name="const", bufs=1))
        big = ctx.enter_context(tc.tile_pool(name="big", bufs=1))
        work = ctx.enter_context(tc.tile_pool(name="work", bufs=1))
        small = ctx.enter_context(tc.tile_pool(name="small", bufs=1))
        stage = ctx.enter_context(tc.tile_pool(name="stage", bufs=8))

        ps_s = ctx.enter_context(tc.tile_pool(name="ps_s", bufs=2, space="PSUM"))
        ps_t = ctx.enter_context(tc.tile_pool(name="ps_t", bufs=2, space="PSUM"))
        ps_o = ctx.enter_context(tc.tile_pool(name="ps_o", bufs=2, space="PSUM"))

        ident = const.tile([P, P], BF16, tag="ident", name="ident")
        make_identity(nc, ident[:])

        for _rep in range(repeat):
            _kernel_body(nc, tc, const, big, work, small, stage,
                         ps_s, ps_t, ps_o, ident,
                         a_dram, v_dram, at_dram, vt_dram, wt_dram, b_dram,
                         out_dram)

    nc.compile()
    _CACHE[("nc", repeat, _SCORES_BANK_INTERLEAVE)] = nc
    return nc


def _kernel_body(nc, tc, const, big, work, small, stage,
                 ps_s, ps_t, ps_o, ident,
                 a_dram, v_dram, at_dram, vt_dram, wt_dram, b_dram, out_dram):
    # ---- load operands (bf16; transposed copies prepared on host) ----
    # Issue order = startup critical path: the first WV matmul needs all of
    # WT plus VT[0]; put those 8 tiles at the head of the DMA queues.
    WT = [big.tile([P, C], BF16, tag=f"WT{cc}", name=f"WT{cc}")
          for cc in range(ND)]
    for cc in range(ND):
        nc.sync.dma_start(WT[cc][:], wt_dram[cc * P:(cc + 1) * P, :])

    VT = [[big.tile([P, T], BF16, tag=f"VT{i}_{cc}", name=f"VT{i}_{cc}")
           for cc in range(ND)] for i in range(3)]
    AT = [[big.tile([P, T], BF16, tag=f"AT{j}_{cc}", name=f"AT{j}_{cc}")
           for cc in range(ND)] for j in range(3)]
    for cc in range(ND):
        nc.sync.dma_start(VT[0][cc][:], vt_dram[0][cc * P:(cc + 1) * P, :])

    b_sb = []
    for dc in range(ND):
        t_ = const.tile([P, 1], F32, tag=f"b{dc}", name=f"b{dc}")
        nc.sync.dma_start(t_[:], b_dram[dc])
        b_sb.append(t_)

    for i in range(1, 3):
        for cc in range(ND):
            nc.sync.dma_start(VT[i][cc][:], vt_dram[i][cc * P:(cc + 1) * P, :])
    for j in range(3):
        for cc in range(ND):
            nc.sync.dma_start(AT[j][cc][:], at_dram[j][cc * P:(cc + 1) * P, :])

    Abf = [[big.tile([P, C], BF16, tag=f"A{j}_{tb}", name=f"A{j}_{tb}")
            for tb in range(NT)] for j in range(3)]
    Vbf = [[big.tile([P, C], BF16, tag=f"V{i}_{tb}", name=f"V{i}_{tb}")
            for tb in range(NT)] for i in range(3)]
    for j in range(3):
        for tb in range(NT):
            nc.sync.dma_start(Abf[j][tb][:], a_dram[j][tb * P:(tb + 1) * P, :])
    for i in range(3):
        for tb in range(NT):
            nc.sync.dma_start(Vbf[i][tb][:], v_dram[i][tb * P:(tb + 1) * P, :])

    # ---- WV^T_i[d, t] = W^T @ V^T_i + b (bf16 out, bias folded in) ----
    # cc outer / th inner: consecutive matmuls share the stationary operand,
    # halving LDWEIGHTS traffic (the two halves accumulate in two banks).
    WVT = [[big.tile([P, T], BF16, tag=f"WVT{i}_{dc}", name=f"WVT{i}_{dc}")
            for dc in range(ND)] for i in range(3)]
    for i in range(3):
        for dc in range(ND):
            po2 = [ps_o.tile([P, C], F32, tag="o", name="o") for _ in range(2)]
            for cc in range(ND):
                for th in range(2):
                    nc.tensor.matmul(po2[th][:], WT[cc][:, dc * P:(dc + 1) * P],
                                     VT[i][cc][:, th * C:(th + 1) * C],
                                     start=(cc == 0), stop=(cc == ND - 1))
            for th in range(2):
                nc.scalar.activation(WVT[i][dc][:, th * C:(th + 1) * C],
                                     po2[th][:], AF.Identity,
                                     bias=b_sb[dc][:], scale=1.0)

    # ---- main loop over the 9 attention pairs ----
    for i in range(3):
        for j in range(3):
            Pt = [work.tile([P, T], BF16, tag=f"P{tb}", name=f"P{tb}")
                  for tb in range(NT)]
            PTa = work.tile([P, NT * T], BF16, tag="PTall", name="PTall")
            PTv = PTa.rearrange("p (sc t) -> p sc t", sc=NT)
            recip = [small.tile([P, 1], F32, tag=f"rc{tb}", name=f"rc{tb}")
                     for tb in range(NT)]
            Vr = [work.tile([P, C], BF16, tag=f"Vr{tb}", name=f"Vr{tb}")
                  for tb in range(NT)]

            def transpose_block(tb):
                # 8 transposed blocks of P[tb] -> one PSUM bank -> strided copy
                pt = ps_t.tile([P, T], BF16, tag="t", name="t")
                for sc in range(NT):
                    nc.tensor.transpose(pt[:, sc * P:(sc + 1) * P],
                                        Pt[tb][:, sc * P:(sc + 1) * P], ident[:])
                nc.vector.tensor_copy(
                    PTv[:, :, tb * P:(tb + 1) * P],
                    pt[:].rearrange("p (sc t) -> p sc t", sc=NT))

            for tb in range(NT):
                # one [128, 1024] score block = 2 PSUM banks; each matmul
                # stays within one bank. dc outer / h inner shares the
                # stationary operand between consecutive matmuls.
                ps = ps_s.tile([P, T], F32, tag="s", name="s")
                if _SCORES_BANK_INTERLEAVE:
                    for dc in range(ND):
                        for h in range(2):
                            nc.tensor.matmul(ps[:, h * C:(h + 1) * C],
                                             WVT[i][dc][:, tb * P:(tb + 1) * P],
                                             AT[j][dc][:, h * C:(h + 1) * C],
                                             start=(dc == 0), stop=(dc == ND - 1))
                else:
                    for h in range(2):
                        for dc in range(ND):
                            nc.tensor.matmul(ps[:, h * C:(h + 1) * C],
                                             WVT[i][dc][:, tb * P:(tb + 1) * P],
                                             AT[j][dc][:, h * C:(h + 1) * C],
                                             start=(dc == 0), stop=(dc == ND - 1))
                rsum = small.tile([P, 1], F32, tag=f"rsum{tb}", name=f"rsum{tb}")
                nc.scalar.activation(Pt[tb][:], ps[:], AF.Exp, scale=SCALE,
                                     accum_out=rsum[:])
                nc.vector.reciprocal(recip[tb][:], rsum[:])
                nc.vector.tensor_scalar_mul(Vr[tb][:], Vbf[i][tb][:],
                                            recip[tb][:])
                if tb >= 1:
                    transpose_block(tb - 1)
            transpose_block(NT - 1)

            # fa[j,i] = P_raw^T @ (diag(recip) @ V_i)
            # fv[i,j] = diag(recip) @ (P_raw @ A_j)
            # interleaved so the kernel tail drains two engines in parallel
            for k in range(NT):
                po = ps_o.tile([P, C], F32, tag="o", name="o")
                for tb in range(NT):
                    nc.tensor.matmul(po[:], Pt[tb][:, k * P:(k + 1) * P],
                                     Vr[tb][:],
                                     start=(tb == 0), stop=(tb == NT - 1))
                st = stage.tile([P, C], BF16, tag="st", name="st")
                nc.vector.tensor_copy(st[:], po[:])
                nc.sync.dma_start(out_dram[1, j, i, k * P:(k + 1) * P, :], st[:])

                po = ps_o.tile([P, C], F32, tag="o", name="o")
                for sc in range(NT):
                    nc.tensor.matmul(
                        po[:], PTa[:, sc * T + k * P: sc * T + (k + 1) * P],
                        Abf[j][sc][:],
                        start=(sc == 0), stop=(sc == NT - 1))
                st = stage.tile([P, C], BF16, tag="st", name="st")
                nc.scalar.activation(st[:], po[:], AF.Copy, bias=0.0,
                                     scale=recip[k][:])
                nc.sync.dma_start(out_dram[0, i, j, k * P:(k + 1) * P, :], st[:])


def _prep_in_maps(a0, a1, a2, v0, v1, v2, W, b):
    bf = ml_dtypes.bfloat16
    a_bf = [np.asarray(x, dtype=np.float32).astype(bf) for x in (a0, a1, a2)]
    v_bf = [np.asarray(x, dtype=np.float32).astype(bf) for x in (v0, v1, v2)]
    wt_bf = np.ascontiguousarray(np.asarray(W, dtype=np.float32).astype(bf).T)
    b_r = np.ascontiguousarray(
        np.asarray(b, dtype=np.float32).reshape(ND, P, 1))
    in_maps = []
    for bi in range(B):
        m = {f"a{j}": np.ascontiguousarray(a_bf[j][bi]) for j in range(3)}
        m.update({f"v{i}": np.ascontiguousarray(v_bf[i][bi]) for i in range(3)})
        m.update({f"at{j}": np.ascontiguousarray(a_bf[j][bi].T)
                  for j in range(3)})
        m.update({f"vt{i}": np.ascontiguousarray(v_bf[i][bi].T)
                  for i in range(3)})
        m["WT"] = wt_bf
        m["bvec"] = b_r
        in_maps.append(m)
    return in_maps


def run(inputs, trace=False, tmpdir=None):
    """Build+run on 8 cores; returns (full_output, BassKernelResults)."""
    nc = _build()
    in_maps = _prep_in_maps(**inputs)
    res = run_bass_kernel_spmd(nc, in_maps, list(range(B)), trace=trace,
                               tmpdir=tmpdir)
    out = np.empty((2, 3, 3, B, T, C), dtype=np.float32)
    for bi in range(B):
        out[:, :, :, bi] = res.results[bi]["out"].astype(np.float32)
    return out, res


def kernel(a0, a1, a2, v0, v1, v2, W, b):
    out, _ = run(dict(a0=a0, a1=a1, a2=a2, v0=v0, v1=v1, v2=v2, W=W, b=b))
    return out



# revision 12
# speedup vs baseline: 528.1759x; 1.2488x over previous
"""Trainium2 Bass kernel for nn_CrossAttention (3x3 scale-grid cross attention).

Reference computation (per batch b):
    WV_i = V_i @ W.T + b                    (video projection, i in 0..2)
    S_ij = (WV_i @ A_j.T) / sqrt(C)         [T, S] scores
    P_ij = softmax(S_ij, axis=-1)
    fv[i,j] = P_ij @ A_j        -> out[0, i, j, b]
    fa[j,i] = P_ij.T @ V_i      -> out[1, j, i, b]

Sharding: data-parallel over batch B=8 across the 8 NeuronCores (one batch
element per core). W/b replicated. Each core runs all 9 (i,j) attention pairs
for its batch element.

On-chip plan (per core): bf16 matmul paths with fp32 PSUM accumulation.
Softmax normalization is folded into the outputs (fv scaled at the PSUM->SBUF
copy, fa via a row-scaled copy of V). The host pre-transposes A/V/W (layout
prep only) so the device only transposes P. P^T transposes are software-
pipelined into the scores phase: the 8 transposed blocks of each P row-block
go to one PSUM bank and leave via one strided DVE copy.
"""

import numpy as np
from contextlib import ExitStack

import ml_dtypes

import concourse.bacc as bacc
import concourse.bass as bass
import concourse.mybir as mybir
import concourse.tile as tile
from concourse.bass_utils import run_bass_kernel_spmd
from concourse.masks import make_identity

BF16 = mybir.dt.bfloat16
F32 = mybir.dt.float32
AF = mybir.ActivationFunctionType

B, T, C = 8, 1024, 512
P = 128
NT = T // P   # 8 row blocks
ND = C // P   # 4 feature chunks
SCALE = 1.0 / float(np.sqrt(C))

_CACHE = {}

# Consecutive score matmuls share the stationary operand (fewer LDWEIGHTS)
# at the cost of alternating PSUM banks between back-to-back matmuls.
# Measured on HW: bank alternation costs ~2% — keep sequential.
_SCORES_BANK_INTERLEAVE = False


def _build(repeat=1):
    key = ("nc", repeat, _SCORES_BANK_INTERLEAVE)
    if key in _CACHE:
        return _CACHE[key]

    nc = bacc.Bacc("TRN2", target_bir_lowering=False, debug=False, num_devices=8)

    a_dram = [nc.dram_tensor(f"a{j}", [T, C], BF16, kind="ExternalInput").ap()
              for j in range(3)]
    v_dram = [nc.dram_tensor(f"v{i}", [T, C], BF16, kind="ExternalInput").ap()
              for i in range(3)]
    at_dram = [nc.dram_tensor(f"at{j}", [C, T], BF16, kind="ExternalInput").ap()
               for j in range(3)]
    vt_dram = [nc.dram_tensor(f"vt{i}", [C, T], BF16, kind="ExternalInput").ap()
               for i in range(3)]
    wt_dram = nc.dram_tensor("WT", [C, C], BF16, kind="ExternalInput").ap()
    b_dram = nc.dram_tensor("bvec", [ND, P, 1], F32, kind="ExternalInput").ap()
    out_dram = nc.dram_tensor("out", [2, 3, 3, T, C], BF16, kind="ExternalOutput").ap()

    with ExitStack() as ctx:
        tc = ctx.enter_context(tile.TileContext(nc))

        const = ctx.enter_context(tc.tile_pool(name="const", bufs=1))
        big = ctx.enter_context(tc.tile_pool(name="big", bufs=1))
        work = ctx.enter_context(tc.tile_pool(name="work", bufs=1))
        small = ctx.enter_context(tc.tile_pool(name="small", bufs=1))
        stage = ctx.enter_context(tc.tile_pool(name="stage", bufs=2))

        ps_s = ctx.enter_context(tc.tile_pool(name="ps_s", bufs=2, space="PSUM"))
        ps_t = ctx.enter_context(tc.tile_pool(name="ps_t", bufs=2, space="PSUM"))
        ps_o = ctx.enter_context(tc.tile_pool(name="ps_o", bufs=2, space="PSUM"))

        ident = const.tile([P, P], BF16, tag="ident", name="ident")
        make_identity(nc, ident[:])

        for _rep in range(repeat):
            _kernel_body(nc, tc, const, big, work, small, stage,
                         ps_s, ps_t, ps_o, ident,
                         a_dram, v_dram, at_dram, vt_dram, wt_dram, b_dram,
                         out_dram)

    nc.compile()
    _CACHE[("nc", repeat, _SCORES_BANK_INTERLEAVE)] = nc
    return nc


def _kernel_body(nc, tc, const, big, work, small, stage,
                 ps_s, ps_t, ps_o, ident,
                 a_dram, v_dram, at_dram, vt_dram, wt_dram, b_dram, out_dram):
    # ---- load operands (bf16; transposed copies prepared on host) ----
    # Issue order = startup critical path: the first WV matmul needs all of
    # WT plus VT[0]; put those 8 tiles at the head of the DMA queues.
    WT = [big.tile([P, C], BF16, tag=f"WT{cc}", name=f"WT{cc}")
          for cc in range(ND)]
    for cc in range(ND):
        nc.sync.dma_start(WT[cc][:], wt_dram[cc * P:(cc + 1) * P, :])

    VT = [[big.tile([P, T], BF16, tag=f"VT{i}_{cc}", name=f"VT{i}_{cc}")
           for cc in range(ND)] for i in range(3)]
    AT = [[big.tile([P, T], BF16, tag=f"AT{j}_{cc}", name=f"AT{j}_{cc}")
           for cc in range(ND)] for j in range(3)]
    for cc in range(ND):
        nc.sync.dma_start(VT[0][cc][:], vt_dram[0][cc * P:(cc + 1) * P, :])

    b_sb = []
    for dc in range(ND):
        t_ = const.tile([P, 1], F32, tag=f"b{dc}", name=f"b{dc}")
        nc.sync.dma_start(t_[:], b_dram[dc])
        b_sb.append(t_)

    for i in range(1, 3):
        for cc in range(ND):
            nc.sync.dma_start(VT[i][cc][:], vt_dram[i][cc * P:(cc + 1) * P, :])
    for j in range(3):
        for cc in range(ND):
            nc.sync.dma_start(AT[j][cc][:], at_dram[j][cc * P:(cc + 1) * P, :])

    Abf = [[big.tile([P, C], BF16, tag=f"A{j}_{tb}", name=f"A{j}_{tb}")
            for tb in range(NT)] for j in range(3)]
    Vbf = [[big.tile([P, C], BF16, tag=f"V{i}_{tb}", name=f"V{i}_{tb}")
            for tb in range(NT)] for i in range(3)]
    for j in range(3):
        for tb in range(NT):
            nc.sync.dma_start(Abf[j][tb][:], a_dram[j][tb * P:(tb + 1) * P, :])
    for i in range(3):
        for tb in range(NT):
            nc.sync.dma_start(Vbf[i][tb][:], v_dram[i][tb * P:(tb + 1) * P, :])

    # ---- WV^T_i[d, t] = W^T @ V^T_i + b (bf16 out, bias folded in) ----
    # cc outer / th inner: consecutive matmuls share the stationary operand,
    # halving LDWEIGHTS traffic (the two halves accumulate in two banks).
    WVT = [[big.tile([P, T], BF16, tag=f"WVT{i}_{dc}", name=f"WVT{i}_{dc}")
            for dc in range(ND)] for i in range(3)]
    for i in range(3):
        for dc in range(ND):
            po2 = [ps_o.tile([P, C], F32, tag="o", name="o") for _ in range(2)]
            for cc in range(ND):
                for th in range(2):
                    nc.tensor.matmul(po2[th][:], WT[cc][:, dc * P:(dc + 1) * P],
                                     VT[i][cc][:, th * C:(th + 1) * C],
                                     start=(cc == 0), stop=(cc == ND - 1))
            for th in range(2):
                nc.scalar.activation(WVT[i][dc][:, th * C:(th + 1) * C],
                                     po2[th][:], AF.Identity,
                                     bias=b_sb[dc][:], scale=1.0)

    # ---- main loop over the 9 attention pairs ----
    for i in range(3):
        for j in range(3):
            Pt = [work.tile([P, T], BF16, tag=f"P{tb}", name=f"P{tb}")
                  for tb in range(NT)]
            PTa = work.tile([P, NT * T], BF16, tag="PTall", name="PTall")
            PTv = PTa.rearrange("p (sc t) -> p sc t", sc=NT)
            recip = [small.tile([P, 1], F32, tag=f"rc{tb}", name=f"rc{tb}")
                     for tb in range(NT)]
            Vr = [work.tile([P, C], BF16, tag=f"Vr{tb}", name=f"Vr{tb}")
                  for tb in range(NT)]

            def transpose_block(tb):
                # 8 transposed blocks of P[tb] -> one PSUM bank -> strided copy
                pt = ps_t.tile([P, T], BF16, tag="t", name="t")
                for sc in range(NT):
                    nc.tensor.transpose(pt[:, sc * P:(sc + 1) * P],
                                        Pt[tb][:, sc * P:(sc + 1) * P], ident[:])
                nc.vector.tensor_copy(
                    PTv[:, :, tb * P:(tb + 1) * P],
                    pt[:].rearrange("p (sc t) -> p sc t", sc=NT))

            for tb in range(NT):
                # one [128, 1024] score block = 2 PSUM banks; each matmul
                # stays within one bank. dc outer / h inner shares the
                # stationary operand between consecutive matmuls.
                ps = ps_s.tile([P, T], F32, tag="s", name="s")
                if _SCORES_BANK_INTERLEAVE:
                    for dc in range(ND):
                        for h in range(2):
                            nc.tensor.matmul(ps[:, h * C:(h + 1) * C],
                                             WVT[i][dc][:, tb * P:(tb + 1) * P],
                                             AT[j][dc][:, h * C:(h + 1) * C],
                                             start=(dc == 0), stop=(dc == ND - 1))
                else:
                    for h in range(2):
                        for dc in range(ND):
                            nc.tensor.matmul(ps[:, h * C:(h + 1) * C],
                                             WVT[i][dc][:, tb * P:(tb + 1) * P],
                                             AT[j][dc][:, h * C:(h + 1) * C],
                                             start=(dc == 0), stop=(dc == ND - 1))
                rsum = small.tile([P, 1], F32, tag=f"rsum{tb}", name=f"rsum{tb}")
                nc.scalar.activation(Pt[tb][:], ps[:], AF.Exp, scale=SCALE,
                                     accum_out=rsum[:])
                nc.vector.reciprocal(recip[tb][:], rsum[:])
                nc.vector.tensor_scalar_mul(Vr[tb][:], Vbf[i][tb][:],
                                            recip[tb][:])
                if tb >= 1:
                    transpose_block(tb - 1)
            transpose_block(NT - 1)

            # fa[j,i] = P_raw^T @ (diag(recip) @ V_i)
            # fv[i,j] = diag(recip) @ (P_raw @ A_j)
            # interleaved so the kernel tail drains two engines in parallel;
            # k-blocks accumulate in SBUF and leave as one 1MB DMA per tensor
            st_fa = stage.tile([P, NT * C], BF16, tag="stfa", name="stfa")
            st_fv = stage.tile([P, NT * C], BF16, tag="stfv", name="stfv")
            for k in range(NT):
                po = ps_o.tile([P, C], F32, tag="o", name="o")
                for tb in range(NT):
                    nc.tensor.matmul(po[:], Pt[tb][:, k * P:(k + 1) * P],
                                     Vr[tb][:],
                                     start=(tb == 0), stop=(tb == NT - 1))
                nc.vector.tensor_copy(st_fa[:, k * C:(k + 1) * C], po[:])

                po = ps_o.tile([P, C], F32, tag="o", name="o")
                for sc in range(NT):
                    nc.tensor.matmul(
                        po[:], PTa[:, sc * T + k * P: sc * T + (k + 1) * P],
                        Abf[j][sc][:],
                        start=(sc == 0), stop=(sc == NT - 1))
                nc.scalar.activation(st_fv[:, k * C:(k + 1) * C], po[:],
                                     AF.Copy, bias=0.0, scale=recip[k][:])
            nc.sync.dma_start(
                out_dram[1, j, i].rearrange("(k p) c -> p k c", k=NT),
                st_fa[:].rearrange("p (k c) -> p k c", k=NT))
            nc.sync.dma_start(
                out_dram[0, i, j].rearrange("(k p) c -> p k c", k=NT),
                st_fv[:].rearrange("p (k c) -> p k c", k=NT))


def _prep_in_maps(a0, a1, a2, v0, v1, v2, W, b):
    bf = ml_dtypes.bfloat16
    a_bf = [np.asarray(x, dtype=np.float32).astype(bf) for x in (a0, a1, a2)]
    v_bf = [np.asarray(x, dtype=np.float32).astype(bf) for x in (v0, v1, v2)]
    wt_bf = np.ascontiguousarray(np.asarray(W, dtype=np.float32).astype(bf).T)
    b_r = np.ascontiguousarray(
        np.asarray(b, dtype=np.float32).reshape(ND, P, 1))
    in_maps = []
    for bi in range(B):
        m = {f"a{j}": np.ascontiguousarray(a_bf[j][bi]) for j in range(3)}
        m.update({f"v{i}": np.ascontiguousarray(v_bf[i][bi]) for i in range(3)})
        m.update({f"at{j}": np.ascontiguousarray(a_bf[j][bi].T)
                  for j in range(3)})
        m.update({f"vt{i}": np.ascontiguousarray(v_bf[i][bi].T)
                  for i in range(3)})
        m["WT"] = wt_bf
        m["bvec"] = b_r
        in_maps.append(m)
    return in_maps


def run(inputs, trace=False, tmpdir=None):
    """Build+run on 8 cores; returns (full_output, BassKernelResults)."""
    nc = _build()
    in_maps = _prep_in_maps(**inputs)
    res = run_bass_kernel_spmd(nc, in_maps, list(range(B)), trace=trace,
                               tmpdir=tmpdir)
    out = np.empty((2, 3, 3, B, T, C), dtype=np.float32)
    for bi in range(B):
        out[:, :, :, bi] = res.results[bi]["out"].astype(np.float32)
    return out, res


def kernel(a0, a1, a2, v0, v1, v2, W, b):
    out, _ = run(dict(a0=a0, a1=a1, a2=a2, v0=v0, v1=v1, v2=v2, W=W, b=b))
    return out

